# revision 17
# baseline (speedup 1.0000x reference)
"""Trainium2 Bass kernel for nn_DeltaEdgeModel (edge-attention GNN).

Strategy (8 NeuronCores, SPMD):
  - Shard the E=4096 query-edge dim of the ExE attention: 512 q-edges/core.
  - Replicate node/edge features + weights; every core computes full K/V.
  - All-transposed layout [dim, edge] on chip so attention needs no
    transposes: scores S^T[k,q] accumulate in PSUM, exp on ScalarE
    (PSUM->SBUF, bf16 out), multiplicative {0,1} adjacency mask on VectorE,
    attn-value matmuls accumulate over k-tiles with 2-head column packing,
    denominators via ones-column matmuls.
  - One AllGather (bf16) between the two attention layers.
Host side does only data layout (gather node features per edge, adjacency
mask, weight re-layout) - all FLOPs run on device.
"""

import sys
import os

for _p in ("/opt/trn_rl_repo", "/root/.axon_site/_ro/trn_rl_repo"):
    if os.path.isdir(_p) and _p not in sys.path:
        sys.path.insert(0, _p)

import numpy as np
import ml_dtypes

import concourse.bass as bass
import concourse.bacc as bacc
import concourse.mybir as mybir
import concourse.tile as tile
from concourse.bass_utils import run_bass_kernel_spmd

BF16 = ml_dtypes.bfloat16
F32 = mybir.dt.float32
BF = mybir.dt.bfloat16
AF = mybir.ActivationFunctionType

N_CORES = 8
N_NODES, E = 1024, 4096
D = 256          # edge dim
H = 4            # heads
HD = 64          # head dim
NCLS = 16
QL = E // N_CORES          # local query edges per core = 512
KT = E // 128              # k tiles = 32
SQ = 1.0 / np.sqrt(HD)     # folded into Wq/bq on host
DEBUG = True


# --------------------------------------------------------------------------
# device program
# --------------------------------------------------------------------------

def build_nc():
    nc = bacc.Bacc("TRN2", target_bir_lowering=False, debug=False,
                   num_devices=N_CORES)

    def din(name, shape, dt=F32):
        return nc.dram_tensor(name, shape, dt, kind="ExternalInput")

    # host-pre-laid-out inputs (per core)
    g_t = din("g_t", [128, 2, E], BF)          # G^T tiles (G = nf[src]||nf[dst])
    ef_t = din("ef_t", [128, 2, E], BF)        # edge_features^T tiles
    ef_loc = din("ef_loc", [128, 2, QL])       # fp32 local slice (residual)
    g_loc = din("g_loc", [128, 2, QL], BF)
    mask = din("mask", [128, KT, QL], BF)      # adjacency {0,1}, k-tiles x local q
    wn = [din(f"w_n{l}", [128, 2, D], BF) for l in (1, 2)]
    wq = [din(f"w_q{l}", [128, 2, D]) for l in (1, 2)]
    wk = [din(f"w_k{l}", [128, 2, D], BF) for l in (1, 2)]
    wv = [din(f"w_v{l}", [128, 2, D], BF) for l in (1, 2)]
    wo = [din(f"w_o{l}", [64, H, D], BF) for l in (1, 2)]
    bn = [din(f"b_n{l}", [128, 2]) for l in (1, 2)]
    bq = [din(f"b_q{l}", [128, 2]) for l in (1, 2)]
    bk = [din(f"b_k{l}", [128, 2]) for l in (1, 2)]
    bo = [din(f"b_o{l}", [128, 2]) for l in (1, 2)]
    wc1 = din("w_c1", [128, 2, D])
    bc1 = din("b_c1", [128, 2])
    wc2 = din("w_c2", [128, 2, NCLS])
    bc2 = din("b_c2", [NCLS, 1])
    id_f = din("id_f", [128, 128])
    id_b = din("id_b", [128, 128], BF)

    out = nc.dram_tensor("out", [QL, NCLS], F32, kind="ExternalOutput")
    dbg = {}
    if DEBUG:
        dbg["x_t"] = nc.dram_tensor("dbg_x_t", [128, 2, E], BF, kind="ExternalOutput")
        dbg["k_t"] = nc.dram_tensor("dbg_k_t", [128, 2, E], BF, kind="ExternalOutput")
        dbg["q_t"] = nc.dram_tensor("dbg_q_t", [128, 2, QL], BF, kind="ExternalOutput")
        dbg["aon"] = nc.dram_tensor("dbg_aon", [64, H, QL], BF, kind="ExternalOutput")
        dbg["o1"] = nc.dram_tensor("dbg_o1", [128, 2, QL], F32, kind="ExternalOutput")

    with tile.TileContext(nc) as tc:
        with (
            tc.tile_pool(name="const", bufs=1) as cp,
            tc.tile_pool(name="work", bufs=1) as wp,
            tc.tile_pool(name="ppool", bufs=3) as ppool,
            tc.tile_pool(name="psproj", bufs=2, space="PSUM") as pp,
            tc.tile_pool(name="psscore", bufs=3, space="PSUM") as pss,
            tc.tile_pool(name="dram", bufs=1, space="DRAM") as dp,
        ):
            def load(dram, shape, dt=F32, pool=cp):
                t = pool.tile(shape, dt, tag=f"c_{dram.name}")
                nc.sync.dma_start(t[:], dram[:])
                return t

            g_t_s = load(g_t, [128, 2, E], BF)
            ef_loc_s = load(ef_loc, [128, 2, QL])
            g_loc_s = load(g_loc, [128, 2, QL], BF)
            mask_s = load(mask, [128, KT, QL], BF)
            wn_s = [load(w, [128, 2, D], BF) for w in wn]
            wq_s = [load(w, [128, 2, D]) for w in wq]
            wk_s = [load(w, [128, 2, D], BF) for w in wk]
            wv_s = [load(w, [128, 2, D], BF) for w in wv]
            wo_s = [load(w, [64, H, D], BF) for w in wo]
            bn_s = [load(b, [128, 2]) for b in bn]
            bq_s = [load(b, [128, 2]) for b in bq]
            bk_s = [load(b, [128, 2]) for b in bk]
            bo_s = [load(b, [128, 2]) for b in bo]
            wc1_s = load(wc1, [128, 2, D])
            bc1_s = load(bc1, [128, 2])
            wc2_s = load(wc2, [128, 2, NCLS])
            bc2_s = load(bc2, [NCLS, 1])
            id_f_s = load(id_f, [128, 128])
            id_b_s = load(id_b, [128, 128], BF)

            # ef_t and the layer-2 gathered activations share one big slot
            ef_t_s = wp.tile([128, 2, E], BF, tag="bigbf")
            nc.sync.dma_start(ef_t_s[:], ef_t[:])

            _dbg_l0 = {}
            mm = nc.tensor.matmul

            def psum_to(dst, ps, bias=None, use_act=True):
                if use_act:
                    if bias is None:
                        nc.scalar.copy(dst, ps)
                    else:
                        nc.scalar.activation(dst, ps, AF.Identity, bias=bias)
                else:
                    if bias is None:
                        nc.vector.tensor_copy(dst, ps)
                    else:
                        nc.vector.tensor_scalar_add(dst, ps, bias)

            def layer(l, src_full, src_loc, prev_loc):
                """src_full: callable (dt, blk)->AP of x-source (bf16) full edges.
                src_loc: fp32 AP fn dt->AP local residual source."""
                W_n, W_q, W_k, W_v, W_o = wn_s[l], wq_s[l], wk_s[l], wv_s[l], wo_s[l]
                # ---- x^T full (bf16: feeds only the K/V projections) ----
                x_t = wp.tile([128, 2, E], BF, tag="x_t")
                for dt in range(2):
                    dsl = slice(dt * 128, dt * 128 + 128)
                    for blk in range(8):
                        bsl = slice(blk * 512, blk * 512 + 512)
                        ps = pp.tile([128, 512], F32, tag="proj")
                        mm(ps[:], W_n[:, 0, dsl], g_t_s[:, 0, bsl], start=True, stop=False)
                        mm(ps[:], W_n[:, 1, dsl], g_t_s[:, 1, bsl], start=False, stop=False)
                        mm(ps[:], id_b_s[:], src_full(dt, blk), start=False, stop=True)
                        psum_to(x_t[:, dt, bsl], ps[:], bn_s[l][:, dt:dt + 1],
                                use_act=(blk % 2 == 0))
                # ---- K^T (bf16) ----
                k_t = wp.tile([128, 2, E], BF, tag="k_t")
                for dt in range(2):
                    dsl = slice(dt * 128, dt * 128 + 128)
                    for blk in range(8):
                        bsl = slice(blk * 512, blk * 512 + 512)
                        ps = pp.tile([128, 512], F32, tag="proj")
                        mm(ps[:], W_k[:, 0, dsl], x_t[:, 0, bsl], start=True, stop=False)
                        mm(ps[:], W_k[:, 1, dsl], x_t[:, 1, bsl], start=False, stop=True)
                        psum_to(k_t[:, dt, bsl], ps[:], bk_s[l][:, dt:dt + 1],
                                use_act=(blk % 2 == 1))
                # ---- x^T local (fp32, exact residual) ----
                xloc = wp.tile([128, 2, QL], F32, tag="xloc")
                for dt in range(2):
                    dsl = slice(dt * 128, dt * 128 + 128)
                    ps = pp.tile([128, 512], F32, tag="proj")
                    mm(ps[:], W_n[:, 0, dsl], g_loc_s[:, 0, :], start=True, stop=False)
                    mm(ps[:], W_n[:, 1, dsl], g_loc_s[:, 1, :], start=False, stop=False)
                    mm(ps[:], id_f_s[:], src_loc(dt), start=False, stop=True)
                    psum_to(xloc[:, dt, :], ps[:], bn_s[l][:, dt:dt + 1])
                # ---- Q^T local (bf16) ----
                q_t = wp.tile([128, 2, QL], BF, tag="q_t")
                for dt in range(2):
                    dsl = slice(dt * 128, dt * 128 + 128)
                    ps = pp.tile([128, 512], F32, tag="proj")
                    mm(ps[:], W_q[:, 0, dsl], xloc[:, 0, :], start=True, stop=False)
                    mm(ps[:], W_q[:, 1, dsl], xloc[:, 1, :], start=False, stop=True)
                    psum_to(q_t[:, dt, :], ps[:], bq_s[l][:, dt:dt + 1])
                # ---- V (normal [e, d] layout, bf16), 65th column = ones so
                # the attn@v matmul also produces the softmax denominator.
                # bv is folded into bo on the host (attn rows sum to 1).
                v_s = wp.tile([128, KT, H, HD + 1], BF, tag="v")
                # contiguous memset; V copies then overwrite cols 0:64 of each
                # head, leaving col 64 == 1.0 (denominator ones column)
                nc.vector.memset(v_s[:], 1.0)
                for i in range(16):
                    ps = pp.tile([128, 512], F32, tag="proj")
                    for half in range(2):
                        et = 2 * i + half
                        esl = slice(et * 128, et * 128 + 128)
                        osl = slice(half * 256, half * 256 + 256)
                        # one accumulation group per PSUM bank: single
                        # start (bank-wide has_written clear) / single stop
                        mm(ps[:, osl], x_t[:, 0, esl], W_v[:, 0, :],
                           start=(half == 0), stop=False)
                        mm(ps[:, osl], x_t[:, 1, esl], W_v[:, 1, :],
                           start=False, stop=(half == 1))
                    for half in range(2):
                        nc.vector.tensor_copy(
                            v_s[:, 2 * i + half, :, 0:HD],
                            ps[:, half * 256:half * 256 + 256].rearrange(
                                "p (h d) -> p h d", h=H))
                # ---- attention k-loop, head pairs ----
                aon = wp.tile([64, H, QL], BF, tag="aon")
                for pair in range(2):
                    # per-head accumulator [V_h | ones] -> rows 0:64 attn@v,
                    # row 64 the softmax denominator; one group per bank.
                    pav = [pp.tile([128, 512], F32, tag="proj", name=f"pav{_h}")
                           for _h in range(2)]
                    for kt in range(KT):
                        ksl = slice(kt * 128, kt * 128 + 128)
                        ps_t = pss.tile([128, 2, 512], F32, tag="s")
                        mm(ps_t[:, 0, :], k_t[0:64, pair, ksl], q_t[0:64, pair, :],
                           start=True, stop=True, tile_position=(0, 0))
                        mm(ps_t[:, 1, :], k_t[64:128, pair, ksl], q_t[64:128, pair, :],
                           start=True, stop=True, tile_position=(64, 0))
                        p_t = ppool.tile([128, 2, 512], BF, tag="p")
                        nc.scalar.activation(p_t[:], ps_t[:], AF.Exp)
                        nc.vector.tensor_mul(p_t[:, 0, :], p_t[:, 0, :], mask_s[:, kt, :])
                        nc.vector.tensor_mul(p_t[:, 1, :], p_t[:, 1, :], mask_s[:, kt, :])
                        st, sp = kt == 0, kt == KT - 1
                        for hh in range(2):
                            mm(pav[hh][0:HD + 1, :],
                               v_s[:, kt, 2 * pair + hh, :], p_t[:, hh, :],
                               start=st, stop=sp)
                    # 1/denom as exp(-ln(d)) on ScalarE (DVE reciprocal is
                    # slow; reciprocal_approx_fast and partition_broadcast
                    # from partitions>0 are broken on hw). The recip lands on
                    # partition 64; DMA it to partition 0, then broadcast.
                    rcp = wp.tile([128, 2, 512], F32, tag="rcp")
                    rb0 = wp.tile([1, 2, 512], F32, tag="rb0")
                    rb = wp.tile([128, 2, 512], F32, tag="rb")
                    for hh in range(2):
                        nc.scalar.activation(rcp[HD:HD + 1, hh, :],
                                             pav[hh][HD:HD + 1, :], AF.Ln)
                        nc.scalar.activation(rcp[HD:HD + 1, hh, :],
                                             rcp[HD:HD + 1, hh, :], AF.Exp,
                                             scale=-1.0)
                    nc.sync.dma_start(rb0[0:1, :, :], rcp[HD:HD + 1, :, :])
                    for hh in range(2):
                        nc.gpsimd.partition_broadcast(
                            rb[0:HD, hh, :], rb0[0:1, hh, :])
                        nc.vector.tensor_mul(aon[0:HD, 2 * pair + hh, :],
                                             pav[hh][0:HD, :], rb[0:HD, hh, :])
                # ---- y = aon @ Wo + bo + x(residual) ----
                if DEBUG and l == 0:
                    _dbg_l0.update(x_t=x_t, k_t=k_t, q_t=q_t, aon=aon)
                oloc = wp.tile([128, 2, QL], F32, tag=f"outloc{l}")
                for et in range(2):
                    esl = slice(et * 128, et * 128 + 128)
                    ps = pp.tile([128, 512], F32, tag="proj")
                    for h in range(H):
                        mm(ps[:], W_o[0:HD, h, esl], aon[0:HD, h, :],
                           start=(h == 0), stop=False)
                    mm(ps[:], id_f_s[:], xloc[:, et, :], start=False, stop=True)
                    psum_to(oloc[:, et, :], ps[:], bo_s[l][:, et:et + 1])
                return oloc

            # ============ layer 1 ============
            o1loc = layer(0,
                          lambda dt, blk: ef_t_s[:, dt, blk * 512:blk * 512 + 512],
                          lambda dt: ef_loc_s[:, dt, :],
                          None)

            if DEBUG:
                for nmm, tt in (("x_t", _dbg_l0["x_t"]), ("k_t", _dbg_l0["k_t"]),
                                ("q_t", _dbg_l0["q_t"]), ("aon", _dbg_l0["aon"]),
                                ("o1", o1loc)):
                    nc.sync.dma_start(dbg[nmm][:], tt[:])

            # ============ exchange: AllGather local out1 (bf16) ============
            o1bf = wp.tile([128, 2, QL], BF, tag="o1bf")
            nc.vector.tensor_copy(o1bf[:], o1loc[:])
            cc_in = dp.tile([2, 128, QL], BF)
            cc_out = dp.tile([N_CORES, 2, 128, QL], BF)
            nc.sync.dma_start(cc_in[:].rearrange("t p q -> p t q"), o1bf[:])
            nc.gpsimd.collective_compute(
                "AllGather",
                mybir.AluOpType.bypass,
                replica_groups=[list(range(N_CORES))],
                ins=[cc_in[:].opt()],
                outs=[cc_out[:].opt()],
            )
            o1g = wp.tile([128, N_CORES, 2, QL], BF, tag="bigbf")
            nc.sync.dma_start(o1g[:], cc_out[:].rearrange("c t p q -> p c t q"))

            # ============ layer 2 ============
            o2loc = layer(1,
                          lambda dt, blk: o1g[:, blk, dt, :],
                          lambda dt: o1loc[:, dt, :],
                          o1loc)

            # ============ classifier ============
            h_s = wp.tile([128, 2, QL], F32, tag="h")
            for dt in range(2):
                dsl = slice(dt * 128, dt * 128 + 128)
                ps = pp.tile([128, 512], F32, tag="proj")
                mm(ps[:], wc1_s[:, 0, dsl], o2loc[:, 0, :], start=True, stop=False)
                mm(ps[:], wc1_s[:, 1, dsl], o2loc[:, 1, :], start=False, stop=True)
                nc.scalar.activation(h_s[:, dt, :], ps[:], AF.Gelu,
                                     bias=bc1_s[:, dt:dt + 1])
            ps_l = pp.tile([128, 512], F32, tag="proj")
            mm(ps_l[0:NCLS, :], wc2_s[:, 0, :], h_s[:, 0, :], start=True, stop=False)
            mm(ps_l[0:NCLS, :], wc2_s[:, 1, :], h_s[:, 1, :], start=False, stop=True)
            lg = wp.tile([NCLS, QL], F32, tag="lg")
            nc.scalar.activation(lg[:], ps_l[0:NCLS, :], AF.Identity,
                                 bias=bc2_s[:, 0:1])
            out_s = wp.tile([128, 4, NCLS], F32, tag="outs")
            for qt in range(4):
                ps = pp.tile([128, 512], F32, tag="proj")
                nc.tensor.transpose(ps[0:128, 0:NCLS],
                                    lg[0:NCLS, qt * 128:qt * 128 + 128],
                                    id_f_s[0:NCLS, 0:NCLS])
                nc.vector.tensor_copy(out_s[:, qt, :], ps[0:128, 0:NCLS])
            nc.sync.dma_start(out[:].rearrange("(qt p) j -> p qt j", p=128), out_s[:])

    nc.compile()
    return nc


# --------------------------------------------------------------------------
# host-side data prep
# --------------------------------------------------------------------------

def _tiles_T(a):
    """[E, D2] array -> transposed tile layout [128, D2//128, E]."""
    d2 = a.shape[1]
    return np.ascontiguousarray(
        a.T.reshape(d2 // 128, 128, a.shape[0]).transpose(1, 0, 2))


def _wtile(w):
    """[G, D] weight -> [128, G//128, D] (lhsT tiles, partition=contraction)."""
    g, d = w.shape
    return np.ascontiguousarray(w.reshape(g // 128, 128, d).transpose(1, 0, 2))


def _btile(b):
    return np.ascontiguousarray(b.reshape(-1, 128).T)  # [128, 2]


def prep_in_maps(inputs):
    f32 = np.float32
    nf = np.asarray(inputs["node_features"], f32)
    ef = np.asarray(inputs["edge_features"], f32)
    ei = np.asarray(inputs["edge_index"], np.int32)
    src, dst = ei[0], ei[1]

    G = np.concatenate([nf[src], nf[dst]], axis=1)            # [E, 256]
    g_t = _tiles_T(G).astype(BF16)
    ef_t_f = _tiles_T(ef)                                      # [128, 2, E] f32
    ef_t = ef_t_f.astype(BF16)

    adj = ((src[:, None] == src[None, :]) | (src[:, None] == dst[None, :]) |
           (dst[:, None] == src[None, :]) | (dst[:, None] == dst[None, :]))
    adj_t = adj.reshape(KT, 128, E).transpose(1, 0, 2)         # [128, KT, E]

    com = {}
    for l, pre in ((1, "a1"), (2, "a2")):
        com[f"w_n{l}"] = _wtile(np.asarray(inputs[f"{pre}_Wn"], f32)).astype(BF16)
        com[f"w_q{l}"] = _wtile(np.asarray(inputs[f"{pre}_Wq"], f32) * SQ)
        com[f"w_k{l}"] = _wtile(np.asarray(inputs[f"{pre}_Wk"], f32)).astype(BF16)
        com[f"w_v{l}"] = _wtile(np.asarray(inputs[f"{pre}_Wv"], f32)).astype(BF16)
        Wo = np.asarray(inputs[f"{pre}_Wo"], f32)
        # [64, H, D]: head h rows at partitions 0:64
        com[f"w_o{l}"] = np.ascontiguousarray(
            Wo.reshape(H, HD, D).transpose(1, 0, 2)).astype(BF16)
        com[f"b_n{l}"] = _btile(np.asarray(inputs[f"{pre}_bn"], f32))
        com[f"b_q{l}"] = _btile(np.asarray(inputs[f"{pre}_bq"], f32) * SQ)
        com[f"b_k{l}"] = _btile(np.asarray(inputs[f"{pre}_bk"], f32))
        # attention rows sum to 1 => the value bias passes through attn@v;
        # fold it into the output-projection bias.
        bo_eff = (np.asarray(inputs[f"{pre}_bo"], f32) +
                  np.asarray(inputs[f"{pre}_bv"], f32) @ Wo)
        com[f"b_o{l}"] = _btile(bo_eff)
    com["w_c1"] = _wtile(np.asarray(inputs["cls_W1"], f32))
    com["b_c1"] = _btile(np.asarray(inputs["cls_b1"], f32))
    com["w_c2"] = _wtile(np.asarray(inputs["cls_W2"], f32))
    com["b_c2"] = np.asarray(inputs["cls_b2"], f32).reshape(NCLS, 1)
    com["id_f"] = np.eye(128, dtype=f32)
    com["id_b"] = np.eye(128, dtype=f32).astype(BF16)
    com["g_t"] = g_t
    com["ef_t"] = ef_t

    in_maps = []
    for c in range(N_CORES):
        q = slice(c * QL, (c + 1) * QL)
        m = dict(com)
        m["ef_loc"] = np.ascontiguousarray(ef_t_f[:, :, q])
        m["g_loc"] = np.ascontiguousarray(g_t[:, :, q])
        m["mask"] = np.ascontiguousarray(adj_t[:, :, q]).astype(BF16)
        in_maps.append(m)
    return in_maps


_NC_CACHE = None


def kernel(**inputs) -> np.ndarray:
    global _NC_CACHE
    in_maps = prep_in_maps(inputs)
    if _NC_CACHE is None:
        _NC_CACHE = build_nc()
    res = run_bass_kernel_spmd(_NC_CACHE, in_maps, core_ids=list(range(N_CORES)))
    return np.concatenate([res.results[c]["out"] for c in range(N_CORES)], axis=0)


# revision 18
# speedup vs baseline: 1.0152x; 1.0152x over previous
"""Trainium2 Bass kernel for nn_DeltaEdgeModel (edge-attention GNN).

Strategy (8 NeuronCores, SPMD):
  - Shard the E=4096 query-edge dim of the ExE attention: 512 q-edges/core.
  - Replicate node/edge features + weights; every core computes full K/V.
  - All-transposed layout [dim, edge] on chip so attention needs no
    transposes: scores S^T[k,q] accumulate in PSUM, exp on ScalarE
    (PSUM->SBUF, bf16 out), multiplicative {0,1} adjacency mask on VectorE,
    attn-value matmuls accumulate over k-tiles with 2-head column packing,
    denominators via ones-column matmuls.
  - One AllGather (bf16) between the two attention layers.
Host side does only data layout (gather node features per edge, adjacency
mask, weight re-layout) - all FLOPs run on device.
"""

import sys
import os

for _p in ("/opt/trn_rl_repo", "/root/.axon_site/_ro/trn_rl_repo"):
    if os.path.isdir(_p) and _p not in sys.path:
        sys.path.insert(0, _p)

import numpy as np
import ml_dtypes

import concourse.bass as bass
import concourse.bacc as bacc
import concourse.mybir as mybir
import concourse.tile as tile
from concourse.bass_utils import run_bass_kernel_spmd

BF16 = ml_dtypes.bfloat16
F32 = mybir.dt.float32
BF = mybir.dt.bfloat16
AF = mybir.ActivationFunctionType

N_CORES = 8
N_NODES, E = 1024, 4096
D = 256          # edge dim
H = 4            # heads
HD = 64          # head dim
NCLS = 16
QL = E // N_CORES          # local query edges per core = 512
KT = E // 128              # k tiles = 32
SQ = 1.0 / np.sqrt(HD)     # folded into Wq/bq on host
DEBUG = False


# --------------------------------------------------------------------------
# device program
# --------------------------------------------------------------------------

def build_nc():
    nc = bacc.Bacc("TRN2", target_bir_lowering=False, debug=False,
                   num_devices=N_CORES)

    def din(name, shape, dt=F32):
        return nc.dram_tensor(name, shape, dt, kind="ExternalInput")

    # host-pre-laid-out inputs (per core)
    g_t = din("g_t", [128, 2, E], BF)          # G^T tiles (G = nf[src]||nf[dst])
    ef_t = din("ef_t", [128, 2, E], BF)        # edge_features^T tiles
    ef_loc = din("ef_loc", [128, 2, QL])       # fp32 local slice (residual)
    g_loc = din("g_loc", [128, 2, QL], BF)
    mask = din("mask", [128, KT, QL], BF)      # adjacency {0,1}, k-tiles x local q
    wn = [din(f"w_n{l}", [128, 2, D], BF) for l in (1, 2)]
    wq = [din(f"w_q{l}", [128, 2, D]) for l in (1, 2)]
    wk = [din(f"w_k{l}", [128, 2, D], BF) for l in (1, 2)]
    wv = [din(f"w_v{l}", [128, 2, D], BF) for l in (1, 2)]
    wo = [din(f"w_o{l}", [64, H, D], BF) for l in (1, 2)]
    bn = [din(f"b_n{l}", [128, 2]) for l in (1, 2)]
    bq = [din(f"b_q{l}", [128, 2]) for l in (1, 2)]
    bk = [din(f"b_k{l}", [128, 2]) for l in (1, 2)]
    bo = [din(f"b_o{l}", [128, 2]) for l in (1, 2)]
    wc1 = din("w_c1", [128, 2, D])
    bc1 = din("b_c1", [128, 2])
    wc2 = din("w_c2", [128, 2, NCLS])
    bc2 = din("b_c2", [NCLS, 1])
    id_f = din("id_f", [128, 128])
    id_b = din("id_b", [128, 128], BF)

    out = nc.dram_tensor("out", [QL, NCLS], F32, kind="ExternalOutput")
    dbg = {}
    if DEBUG:
        dbg["x_t"] = nc.dram_tensor("dbg_x_t", [128, 2, E], BF, kind="ExternalOutput")
        dbg["k_t"] = nc.dram_tensor("dbg_k_t", [128, 2, E], BF, kind="ExternalOutput")
        dbg["q_t"] = nc.dram_tensor("dbg_q_t", [128, 2, QL], BF, kind="ExternalOutput")
        dbg["aon"] = nc.dram_tensor("dbg_aon", [64, H, QL], BF, kind="ExternalOutput")
        dbg["o1"] = nc.dram_tensor("dbg_o1", [128, 2, QL], F32, kind="ExternalOutput")

    with tile.TileContext(nc) as tc:
        with (
            tc.tile_pool(name="const", bufs=1) as cp,
            tc.tile_pool(name="work", bufs=1) as wp,
            tc.tile_pool(name="ppool", bufs=3) as ppool,
            tc.tile_pool(name="psproj", bufs=2, space="PSUM") as pp,
            tc.tile_pool(name="psscore", bufs=3, space="PSUM") as pss,
            tc.tile_pool(name="dram", bufs=1, space="DRAM") as dp,
        ):
            def load(dram, shape, dt=F32, pool=cp):
                t = pool.tile(shape, dt, tag=f"c_{dram.name}")
                nc.sync.dma_start(t[:], dram[:])
                return t

            g_t_s = load(g_t, [128, 2, E], BF)
            ef_loc_s = load(ef_loc, [128, 2, QL])
            g_loc_s = load(g_loc, [128, 2, QL], BF)
            mask_s = load(mask, [128, KT, QL], BF)
            wn_s = [load(w, [128, 2, D], BF) for w in wn]
            wq_s = [load(w, [128, 2, D]) for w in wq]
            wk_s = [load(w, [128, 2, D], BF) for w in wk]
            wv_s = [load(w, [128, 2, D], BF) for w in wv]
            wo_s = [load(w, [64, H, D], BF) for w in wo]
            bn_s = [load(b, [128, 2]) for b in bn]
            bq_s = [load(b, [128, 2]) for b in bq]
            bk_s = [load(b, [128, 2]) for b in bk]
            bo_s = [load(b, [128, 2]) for b in bo]
            wc1_s = load(wc1, [128, 2, D])
            bc1_s = load(bc1, [128, 2])
            wc2_s = load(wc2, [128, 2, NCLS])
            bc2_s = load(bc2, [NCLS, 1])
            id_f_s = load(id_f, [128, 128])
            id_b_s = load(id_b, [128, 128], BF)

            # ef_t and the layer-2 gathered activations share one big slot
            ef_t_s = wp.tile([128, 2, E], BF, tag="bigbf")
            nc.sync.dma_start(ef_t_s[:], ef_t[:])

            _dbg_l0 = {}
            mm = nc.tensor.matmul

            def psum_to(dst, ps, bias=None, use_act=True):
                if use_act:
                    if bias is None:
                        nc.scalar.copy(dst, ps)
                    else:
                        nc.scalar.activation(dst, ps, AF.Identity, bias=bias)
                else:
                    if bias is None:
                        nc.vector.tensor_copy(dst, ps)
                    else:
                        nc.vector.tensor_scalar_add(dst, ps, bias)

            def layer(l, src_full, src_loc, prev_loc):
                """src_full: callable (dt, blk)->AP of x-source (bf16) full edges.
                src_loc: fp32 AP fn dt->AP local residual source."""
                W_n, W_q, W_k, W_v, W_o = wn_s[l], wq_s[l], wk_s[l], wv_s[l], wo_s[l]
                # ---- x^T full (bf16: feeds only the K/V projections) ----
                x_t = wp.tile([128, 2, E], BF, tag="x_t")
                for dt in range(2):
                    dsl = slice(dt * 128, dt * 128 + 128)
                    for blk in range(8):
                        bsl = slice(blk * 512, blk * 512 + 512)
                        ps = pp.tile([128, 512], F32, tag="proj")
                        mm(ps[:], W_n[:, 0, dsl], g_t_s[:, 0, bsl], start=True, stop=False)
                        mm(ps[:], W_n[:, 1, dsl], g_t_s[:, 1, bsl], start=False, stop=False)
                        mm(ps[:], id_b_s[:], src_full(dt, blk), start=False, stop=True)
                        psum_to(x_t[:, dt, bsl], ps[:], bn_s[l][:, dt:dt + 1],
                                use_act=(blk % 2 == 0))
                # ---- K^T (bf16) ----
                k_t = wp.tile([128, 2, E], BF, tag="k_t")
                for dt in range(2):
                    dsl = slice(dt * 128, dt * 128 + 128)
                    for blk in range(8):
                        bsl = slice(blk * 512, blk * 512 + 512)
                        ps = pp.tile([128, 512], F32, tag="proj")
                        mm(ps[:], W_k[:, 0, dsl], x_t[:, 0, bsl], start=True, stop=False)
                        mm(ps[:], W_k[:, 1, dsl], x_t[:, 1, bsl], start=False, stop=True)
                        psum_to(k_t[:, dt, bsl], ps[:], bk_s[l][:, dt:dt + 1],
                                use_act=(blk % 2 == 1))
                # ---- x^T local (fp32, exact residual) ----
                xloc = wp.tile([128, 2, QL], F32, tag="xloc")
                for dt in range(2):
                    dsl = slice(dt * 128, dt * 128 + 128)
                    ps = pp.tile([128, 512], F32, tag="proj")
                    mm(ps[:], W_n[:, 0, dsl], g_loc_s[:, 0, :], start=True, stop=False)
                    mm(ps[:], W_n[:, 1, dsl], g_loc_s[:, 1, :], start=False, stop=False)
                    mm(ps[:], id_f_s[:], src_loc(dt), start=False, stop=True)
                    psum_to(xloc[:, dt, :], ps[:], bn_s[l][:, dt:dt + 1])
                # ---- Q^T local (bf16) ----
                q_t = wp.tile([128, 2, QL], BF, tag="q_t")
                for dt in range(2):
                    dsl = slice(dt * 128, dt * 128 + 128)
                    ps = pp.tile([128, 512], F32, tag="proj")
                    mm(ps[:], W_q[:, 0, dsl], xloc[:, 0, :], start=True, stop=False)
                    mm(ps[:], W_q[:, 1, dsl], xloc[:, 1, :], start=False, stop=True)
                    psum_to(q_t[:, dt, :], ps[:], bq_s[l][:, dt:dt + 1])
                # ---- V (normal [e, d] layout, bf16), 65th column = ones so
                # the attn@v matmul also produces the softmax denominator.
                # bv is folded into bo on the host (attn rows sum to 1).
                v_s = wp.tile([128, KT, H, HD + 1], BF, tag="v")
                # contiguous memset; V copies then overwrite cols 0:64 of each
                # head, leaving col 64 == 1.0 (denominator ones column)
                nc.vector.memset(v_s[:], 1.0)
                for i in range(16):
                    ps = pp.tile([128, 512], F32, tag="proj")
                    for half in range(2):
                        et = 2 * i + half
                        esl = slice(et * 128, et * 128 + 128)
                        osl = slice(half * 256, half * 256 + 256)
                        # one accumulation group per PSUM bank: single
                        # start (bank-wide has_written clear) / single stop
                        mm(ps[:, osl], x_t[:, 0, esl], W_v[:, 0, :],
                           start=(half == 0), stop=False)
                        mm(ps[:, osl], x_t[:, 1, esl], W_v[:, 1, :],
                           start=False, stop=(half == 1))
                    for half in range(2):
                        nc.vector.tensor_copy(
                            v_s[:, 2 * i + half, :, 0:HD],
                            ps[:, half * 256:half * 256 + 256].rearrange(
                                "p (h d) -> p h d", h=H))
                # ---- attention k-loop, head pairs ----
                aon = wp.tile([64, H, QL], BF, tag="aon")
                for pair in range(2):
                    # per-head accumulator [V_h | ones] -> rows 0:64 attn@v,
                    # row 64 the softmax denominator; one group per bank.
                    pav = [pp.tile([128, 512], F32, tag="proj", name=f"pav{_h}")
                           for _h in range(2)]
                    for kt in range(KT):
                        ksl = slice(kt * 128, kt * 128 + 128)
                        ps_t = pss.tile([128, 2, 512], F32, tag="s")
                        mm(ps_t[:, 0, :], k_t[0:64, pair, ksl], q_t[0:64, pair, :],
                           start=True, stop=True, tile_position=(0, 0))
                        mm(ps_t[:, 1, :], k_t[64:128, pair, ksl], q_t[64:128, pair, :],
                           start=True, stop=True, tile_position=(64, 0))
                        p_t = ppool.tile([128, 2, 512], BF, tag="p")
                        nc.scalar.activation(p_t[:], ps_t[:], AF.Exp)
                        nc.vector.tensor_mul(p_t[:, 0, :], p_t[:, 0, :], mask_s[:, kt, :])
                        nc.vector.tensor_mul(p_t[:, 1, :], p_t[:, 1, :], mask_s[:, kt, :])
                        st, sp = kt == 0, kt == KT - 1
                        for hh in range(2):
                            mm(pav[hh][0:HD + 1, :],
                               v_s[:, kt, 2 * pair + hh, :], p_t[:, hh, :],
                               start=st, stop=sp)
                    # 1/denom as exp(-ln(d)) on ScalarE (DVE reciprocal is
                    # slow; reciprocal_approx_fast and partition_broadcast
                    # from partitions>0 are broken on hw). The recip lands on
                    # partition 64; DMA it to partition 0, then broadcast.
                    rcp = wp.tile([128, 2, 512], F32, tag="rcp")
                    rb0 = wp.tile([1, 2, 512], F32, tag="rb0")
                    rb = wp.tile([128, 2, 512], F32, tag="rb")
                    for hh in range(2):
                        nc.scalar.activation(rcp[HD:HD + 1, hh, :],
                                             pav[hh][HD:HD + 1, :], AF.Ln)
                        nc.scalar.activation(rcp[HD:HD + 1, hh, :],
                                             rcp[HD:HD + 1, hh, :], AF.Exp,
                                             scale=-1.0)
                    nc.sync.dma_start(rb0[0:1, :, :], rcp[HD:HD + 1, :, :])
                    for hh in range(2):
                        nc.gpsimd.partition_broadcast(
                            rb[0:HD, hh, :], rb0[0:1, hh, :])
                        nc.vector.tensor_mul(aon[0:HD, 2 * pair + hh, :],
                                             pav[hh][0:HD, :], rb[0:HD, hh, :])
                # ---- y = aon @ Wo + bo + x(residual) ----
                if DEBUG and l == 0:
                    _dbg_l0.update(x_t=x_t, k_t=k_t, q_t=q_t, aon=aon)
                oloc = wp.tile([128, 2, QL], F32, tag=f"outloc{l}")
                for et in range(2):
                    esl = slice(et * 128, et * 128 + 128)
                    ps = pp.tile([128, 512], F32, tag="proj")
                    for h in range(H):
                        mm(ps[:], W_o[0:HD, h, esl], aon[0:HD, h, :],
                           start=(h == 0), stop=False)
                    mm(ps[:], id_f_s[:], xloc[:, et, :], start=False, stop=True)
                    psum_to(oloc[:, et, :], ps[:], bo_s[l][:, et:et + 1])
                return oloc

            # ============ layer 1 ============
            o1loc = layer(0,
                          lambda dt, blk: ef_t_s[:, dt, blk * 512:blk * 512 + 512],
                          lambda dt: ef_loc_s[:, dt, :],
                          None)

            if DEBUG:
                for nmm, tt in (("x_t", _dbg_l0["x_t"]), ("k_t", _dbg_l0["k_t"]),
                                ("q_t", _dbg_l0["q_t"]), ("aon", _dbg_l0["aon"]),
                                ("o1", o1loc)):
                    nc.sync.dma_start(dbg[nmm][:], tt[:])

            # ============ exchange: AllGather local out1 (bf16) ============
            o1bf = wp.tile([128, 2, QL], BF, tag="o1bf")
            nc.vector.tensor_copy(o1bf[:], o1loc[:])
            cc_in = dp.tile([2, 128, QL], BF)
            cc_out = dp.tile([N_CORES, 2, 128, QL], BF)
            nc.sync.dma_start(cc_in[:].rearrange("t p q -> p t q"), o1bf[:])
            nc.gpsimd.collective_compute(
                "AllGather",
                mybir.AluOpType.bypass,
                replica_groups=[list(range(N_CORES))],
                ins=[cc_in[:].opt()],
                outs=[cc_out[:].opt()],
            )
            o1g = wp.tile([128, N_CORES, 2, QL], BF, tag="bigbf")
            nc.sync.dma_start(o1g[:], cc_out[:].rearrange("c t p q -> p c t q"))

            # ============ layer 2 ============
            o2loc = layer(1,
                          lambda dt, blk: o1g[:, blk, dt, :],
                          lambda dt: o1loc[:, dt, :],
                          o1loc)

            # ============ classifier ============
            h_s = wp.tile([128, 2, QL], F32, tag="h")
            for dt in range(2):
                dsl = slice(dt * 128, dt * 128 + 128)
                ps = pp.tile([128, 512], F32, tag="proj")
                mm(ps[:], wc1_s[:, 0, dsl], o2loc[:, 0, :], start=True, stop=False)
                mm(ps[:], wc1_s[:, 1, dsl], o2loc[:, 1, :], start=False, stop=True)
                nc.scalar.activation(h_s[:, dt, :], ps[:], AF.Gelu,
                                     bias=bc1_s[:, dt:dt + 1])
            ps_l = pp.tile([128, 512], F32, tag="proj")
            mm(ps_l[0:NCLS, :], wc2_s[:, 0, :], h_s[:, 0, :], start=True, stop=False)
            mm(ps_l[0:NCLS, :], wc2_s[:, 1, :], h_s[:, 1, :], start=False, stop=True)
            lg = wp.tile([NCLS, QL], F32, tag="lg")
            nc.scalar.activation(lg[:], ps_l[0:NCLS, :], AF.Identity,
                                 bias=bc2_s[:, 0:1])
            out_s = wp.tile([128, 4, NCLS], F32, tag="outs")
            for qt in range(4):
                ps = pp.tile([128, 512], F32, tag="proj")
                nc.tensor.transpose(ps[0:128, 0:NCLS],
                                    lg[0:NCLS, qt * 128:qt * 128 + 128],
                                    id_f_s[0:NCLS, 0:NCLS])
                nc.vector.tensor_copy(out_s[:, qt, :], ps[0:128, 0:NCLS])
            nc.sync.dma_start(out[:].rearrange("(qt p) j -> p qt j", p=128), out_s[:])

    nc.compile()
    return nc


# --------------------------------------------------------------------------
# host-side data prep
# --------------------------------------------------------------------------

def _tiles_T(a):
    """[E, D2] array -> transposed tile layout [128, D2//128, E]."""
    d2 = a.shape[1]
    return np.ascontiguousarray(
        a.T.reshape(d2 // 128, 128, a.shape[0]).transpose(1, 0, 2))


def _wtile(w):
    """[G, D] weight -> [128, G//128, D] (lhsT tiles, partition=contraction)."""
    g, d = w.shape
    return np.ascontiguousarray(w.reshape(g // 128, 128, d).transpose(1, 0, 2))


def _btile(b):
    return np.ascontiguousarray(b.reshape(-1, 128).T)  # [128, 2]


def prep_in_maps(inputs):
    f32 = np.float32
    nf = np.asarray(inputs["node_features"], f32)
    ef = np.asarray(inputs["edge_features"], f32)
    ei = np.asarray(inputs["edge_index"], np.int32)
    src, dst = ei[0], ei[1]

    G = np.concatenate([nf[src], nf[dst]], axis=1)            # [E, 256]
    g_t = _tiles_T(G).astype(BF16)
    ef_t_f = _tiles_T(ef)                                      # [128, 2, E] f32
    ef_t = ef_t_f.astype(BF16)

    adj = ((src[:, None] == src[None, :]) | (src[:, None] == dst[None, :]) |
           (dst[:, None] == src[None, :]) | (dst[:, None] == dst[None, :]))
    adj_t = adj.reshape(KT, 128, E).transpose(1, 0, 2)         # [128, KT, E]

    com = {}
    for l, pre in ((1, "a1"), (2, "a2")):
        com[f"w_n{l}"] = _wtile(np.asarray(inputs[f"{pre}_Wn"], f32)).astype(BF16)
        com[f"w_q{l}"] = _wtile(np.asarray(inputs[f"{pre}_Wq"], f32) * SQ)
        com[f"w_k{l}"] = _wtile(np.asarray(inputs[f"{pre}_Wk"], f32)).astype(BF16)
        com[f"w_v{l}"] = _wtile(np.asarray(inputs[f"{pre}_Wv"], f32)).astype(BF16)
        Wo = np.asarray(inputs[f"{pre}_Wo"], f32)
        # [64, H, D]: head h rows at partitions 0:64
        com[f"w_o{l}"] = np.ascontiguousarray(
            Wo.reshape(H, HD, D).transpose(1, 0, 2)).astype(BF16)
        com[f"b_n{l}"] = _btile(np.asarray(inputs[f"{pre}_bn"], f32))
        com[f"b_q{l}"] = _btile(np.asarray(inputs[f"{pre}_bq"], f32) * SQ)
        com[f"b_k{l}"] = _btile(np.asarray(inputs[f"{pre}_bk"], f32))
        # attention rows sum to 1 => the value bias passes through attn@v;
        # fold it into the output-projection bias.
        bo_eff = (np.asarray(inputs[f"{pre}_bo"], f32) +
                  np.asarray(inputs[f"{pre}_bv"], f32) @ Wo)
        com[f"b_o{l}"] = _btile(bo_eff)
    com["w_c1"] = _wtile(np.asarray(inputs["cls_W1"], f32))
    com["b_c1"] = _btile(np.asarray(inputs["cls_b1"], f32))
    com["w_c2"] = _wtile(np.asarray(inputs["cls_W2"], f32))
    com["b_c2"] = np.asarray(inputs["cls_b2"], f32).reshape(NCLS, 1)
    com["id_f"] = np.eye(128, dtype=f32)
    com["id_b"] = np.eye(128, dtype=f32).astype(BF16)
    com["g_t"] = g_t
    com["ef_t"] = ef_t

    in_maps = []
    for c in range(N_CORES):
        q = slice(c * QL, (c + 1) * QL)
        m = dict(com)
        m["ef_loc"] = np.ascontiguousarray(ef_t_f[:, :, q])
        m["g_loc"] = np.ascontiguousarray(g_t[:, :, q])
        m["mask"] = np.ascontiguousarray(adj_t[:, :, q]).astype(BF16)
        in_maps.append(m)
    return in_maps


_NC_CACHE = None


def kernel(**inputs) -> np.ndarray:
    global _NC_CACHE
    in_maps = prep_in_maps(inputs)
    if _NC_CACHE is None:
        _NC_CACHE = build_nc()
    res = run_bass_kernel_spmd(_NC_CACHE, in_maps, core_ids=list(range(N_CORES)))
    return np.concatenate([res.results[c]["out"] for c in range(N_CORES)], axis=0)
